# revision 4
# baseline (speedup 1.0000x reference)
"""YOLO-style loss (nn_Loss_90142773608781) on 8 Trainium2 NeuronCores.

Strategy (data-parallel by cell range, per sharding hint):
- Cells (16384*7*7 = 802816 rows of 30 floats) are sharded by batch range:
  core c owns cells [c*100352, (c+1)*100352).
- Dense conf term: host extracts cols {4,9} to a [CELLS,2] bf16 array; each
  core squares+accumulates its slice on ScalarE (one op).
- Targeted terms: targets are sharded by their cell's core. Grid rows are
  fetched with ONE dma_gather per core (fast CounterMachine SWDGE path,
  ~0.34ns/desc) from a bf16 table padded to 64B rows: each 256B gathered
  element covers 4 cells, and targets are grouped by cell%4 into 4
  fixed-capacity groups so the sub-offset is compile-time. Padding slots
  gather a dedicated all-zero row; their only residue, (0-1)^2 = 1 per slot,
  is corrected on the host.
- Class terms: host bakes -2*onehot(cls_t) so cls_r becomes a dot product;
  sum(cls^2) runs on ScalarE with accum. The per-target "+1" constant is
  added on the host.
- Each core writes [128,3] partials (target-loss, conf-sq, cls-sq); host
  reduces and applies the constant corrections.
"""

import sys

if "/opt/trn_rl_repo" not in sys.path:
    sys.path.append("/opt/trn_rl_repo")

import numpy as np
import ml_dtypes

BF16 = ml_dtypes.bfloat16

P = 128
D = 30
GRID = 7
BATCH = 16384
NTGT = 65536
CELLS = BATCH * GRID * GRID          # 802816
CELLS_CORE = CELLS // 8              # 100352
PR_CORE = CELLS_CORE // 4            # 25088 pair rows (4 cells / 256B elem)
ZROW = PR_CORE                       # dedicated zero row for padding slots
WINROWS = PR_CORE + 1
CAP = 2432                           # slots per (core, cell%4) group
NG = 4
NS = CAP * NG                        # 9728 slots per core
NCHUNK = NS // P                     # 76
GC = CAP // P                        # 19 chunks per group
IDXW = NS // 16                      # 608
CONF_W = CELLS_CORE * 2 // P         # 1568

_cache = {}


def _build():
    import concourse.bacc as bacc
    import concourse.tile as tile
    import concourse.mybir as mybir

    F32 = mybir.dt.float32
    BF = mybir.dt.bfloat16
    I16 = mybir.dt.int16
    AL = mybir.AluOpType
    ACT = mybir.ActivationFunctionType
    X = mybir.AxisListType.X

    nc = bacc.Bacc("TRN2", target_bir_lowering=False, debug=False,
                   enable_asserts=False, num_devices=8)
    win = nc.dram_tensor("win", [WINROWS, P], BF, kind="ExternalInput").ap()
    idx = nc.dram_tensor("idx", [P, IDXW], I16, kind="ExternalInput").ap()
    fldf = nc.dram_tensor("fldf", [P, 9 * NCHUNK], mybir.dt.float32,
                          kind="ExternalInput").ap()
    hcls = nc.dram_tensor("hcls", [P, 20 * NCHUNK], BF, kind="ExternalInput").ap()
    conf = nc.dram_tensor("conf", [P, CONF_W], BF, kind="ExternalInput").ap()
    out = nc.dram_tensor("partial", [P, 3], mybir.dt.float32,
                         kind="ExternalOutput").ap()

    vec, act = nc.vector, nc.scalar

    with tile.TileContext(nc) as tc:
        with (
            tc.tile_pool(name="io", bufs=1) as io,
            tc.tile_pool(name="scr", bufs=2) as scr,
        ):
            # ---- setup loads (idx first: gather depends on it) ----
            idx_t = io.tile([P, IDXW], I16)
            nc.sync.dma_start(out=idx_t[:], in_=idx[:])

            g = io.tile([P, NS], BF)
            nc.gpsimd.dma_gather(
                g[:].rearrange("p (k e) -> p k e", e=P),
                win[:], idx_t[:], NS, NS, P, single_packet=False,
            )

            fld_t = io.tile([P, 9 * NCHUNK], F32)
            nc.sync.dma_start(out=fld_t[:], in_=fldf[:])
            h_t = io.tile([P, 20 * NCHUNK], BF)
            nc.sync.dma_start(out=h_t[:], in_=hcls[:])
            conf_t = io.tile([P, CONF_W], BF)
            nc.sync.dma_start(out=conf_t[:], in_=conf[:])

            eps_t = io.tile([P, 1], F32)
            vec.memset(eps_t[:], 1e-6)
            neg1_t = io.tile([P, 1], F32)
            vec.memset(neg1_t[:], -1.0)
            acc = io.tile([P, 3], F32)

            # ---- dense conf term on ScalarE (overlaps the gather) ----
            confsq = scr.tile([P, CONF_W], BF, tag="confsq")
            act.activation(confsq[:], conf_t[:], ACT.Square,
                           accum_out=acc[:, 1:2])

            # ---- consolidate gathered cells into packed f32 [P,76,30] ----
            G = io.tile([P, NCHUNK * D], F32)
            g3 = g[:].rearrange("p (k e) -> p k e", e=P)
            G3 = G[:].rearrange("p (k c) -> p k c", c=D)
            for m in range(NG):
                act.mul(G3[:, m * GC:(m + 1) * GC, :],
                        g3[:, m * GC:(m + 1) * GC, 32 * m:32 * m + D], 1.0)

            # ---- views ----
            W = NCHUNK
            g5 = G[:].rearrange("p (k b r) -> p k b r", b=6, r=5)
            xy = g5[:, :, 0:2, 0:2]          # [P,W,2box,2xy]
            wh = g5[:, :, 0:2, 2:4]
            cb = g5[:, :, 0:2, 4]            # [P,W,2]
            clsg = G3[:, :, 10:30]           # [P,W,20]

            pairs = fld_t[:, :8 * NCHUNK].rearrange("p (f k c) -> p f k c",
                                                    f=4, c=2)
            XYt = pairs[:, 0]                # [P,W,2]
            LTt = pairs[:, 1].unsqueeze(2).to_broadcast([P, W, 2, 2])
            RBt = pairs[:, 2].unsqueeze(2).to_broadcast([P, W, 2, 2])
            SSQt = pairs[:, 3]
            areab = fld_t[:, 8 * NCHUNK:9 * NCHUNK].rearrange(
                "p k -> p k").unsqueeze(2).to_broadcast([P, W, 2])

            def t4(tag):
                t = scr.tile([P, W * 4], F32, tag=tag, name=tag)
                return t[:].rearrange("p (k b r) -> p k b r", b=2, r=2)

            def t2(tag):
                t = scr.tile([P, W * 2], F32, tag=tag, name=tag)
                return t[:].rearrange("p (k c) -> p k c", c=2)

            def t1(tag):
                return scr.tile([P, W], F32, tag=tag, name=tag)[:]

            # ---- IoU / responsible-box selection ----
            hwh = t4("hwh")
            act.mul(hwh, wh, 3.5)
            lt = t4("lt")
            vec.tensor_tensor(out=lt, in0=xy, in1=hwh, op=AL.subtract)
            rb = t4("rb")
            vec.tensor_tensor(out=rb, in0=xy, in1=hwh, op=AL.add)

            wih = t4("wih")
            vec.tensor_tensor(out=wih, in0=rb, in1=RBt, op=AL.min)
            mx = t4("mx")
            vec.tensor_tensor(out=mx, in0=lt, in1=LTt, op=AL.max)
            vec.tensor_tensor(out=wih, in0=wih, in1=mx, op=AL.subtract)
            vec.tensor_scalar_max(out=wih, in0=wih, scalar1=0.0)

            ain = t2("ain")
            vec.tensor_tensor(out=ain, in0=wih[:, :, :, 0], in1=wih[:, :, :, 1],
                              op=AL.mult)
            atot = t2("atot")
            vec.tensor_tensor(out=atot, in0=wh[:, :, :, 0], in1=wh[:, :, :, 1],
                              op=AL.mult)
            act.mul(atot, atot, 49.0)
            vec.tensor_tensor(out=atot, in0=atot, in1=areab, op=AL.add)
            vec.tensor_tensor(out=atot, in0=atot, in1=ain, op=AL.subtract)

            pred = t2("pred")
            vec.tensor_scalar(out=pred, in0=atot, scalar1=1e-6, scalar2=None,
                              op0=AL.is_gt)
            vec.tensor_scalar_max(out=atot, in0=atot, scalar1=1e-6)
            vec.reciprocal(out=atot, in_=atot)
            iou = t2("iou")
            vec.tensor_tensor(out=iou, in0=ain, in1=atot, op=AL.mult)
            vec.tensor_tensor(out=iou, in0=iou, in1=pred, op=AL.mult)

            sel2 = t2("sel2")
            i1 = iou[:, :, 1:2].to_broadcast([P, W, 2])
            i0 = iou[:, :, 0:1].to_broadcast([P, W, 2])
            vec.tensor_tensor(out=sel2, in0=i1, in1=i0, op=AL.is_gt)

            def pick2(v3, tag):
                t = t2(tag)
                vec.tensor_tensor(out=t, in0=v3[:, :, 1, :], in1=v3[:, :, 0, :],
                                  op=AL.subtract)
                vec.tensor_tensor(out=t, in0=t, in1=sel2, op=AL.mult)
                vec.tensor_tensor(out=t, in0=t, in1=v3[:, :, 0, :], op=AL.add)
                return t

            xyr = pick2(xy, "xyr")
            whr = pick2(wh, "whr")
            cr = t1("cr")
            vec.tensor_tensor(out=cr, in0=cb[:, :, 1], in1=cb[:, :, 0],
                              op=AL.subtract)
            vec.tensor_tensor(out=cr, in0=cr, in1=sel2[:, :, 0], op=AL.mult)
            vec.tensor_tensor(out=cr, in0=cr, in1=cb[:, :, 0], op=AL.add)

            # ---- coord + size terms ----
            dxy = t2("dxy")
            vec.tensor_tensor(out=dxy, in0=XYt, in1=xyr, op=AL.subtract)
            vec.tensor_tensor(out=dxy, in0=dxy, in1=dxy, op=AL.mult)

            sq_ = t2("sq_")
            sg_ = t2("sg_")
            act.activation(sq_, whr, ACT.Abs)
            act.activation(sq_, sq_, ACT.Sqrt, bias=eps_t[:])
            act.activation(sg_, whr, ACT.Sign)
            vec.tensor_tensor(out=sq_, in0=sq_, in1=sg_, op=AL.mult)
            vec.tensor_tensor(out=sq_, in0=SSQt, in1=sq_, op=AL.subtract)
            vec.tensor_tensor(out=sq_, in0=sq_, in1=sq_, op=AL.mult)

            vec.tensor_tensor(out=dxy, in0=dxy, in1=sq_, op=AL.add)
            L = t1("L")
            vec.tensor_reduce(out=L, in_=dxy, axis=X, op=AL.add)
            act.mul(L, L, 5.0)

            # ---- obj terms: + (cr-1)^2 - 0.5*cr^2 ----
            o1 = t1("o1")
            act.activation(o1, cr, ACT.Square, bias=neg1_t[:])
            vec.tensor_tensor(out=L, in0=L, in1=o1, op=AL.add)
            act.activation(o1, cr, ACT.Square)
            vec.tensor_scalar_mul(out=o1, in0=o1, scalar1=0.5)
            vec.tensor_tensor(out=L, in0=L, in1=o1, op=AL.subtract)

            # ---- class terms: -2*cls_sel (host-baked one-hot) + cls^2 ----
            big = scr.tile([P, W * 20], F32, tag="big")
            big3 = big[:].rearrange("p (k c) -> p k c", c=20)
            h3 = h_t[:].rearrange("p (k c) -> p k c", c=20)
            vec.tensor_tensor(out=big3, in0=h3, in1=clsg, op=AL.mult)
            vec.tensor_reduce(out=o1, in_=big3, axis=X, op=AL.add)
            vec.tensor_tensor(out=L, in0=L, in1=o1, op=AL.add)

            clssq = scr.tile([P, W * 20], BF, tag="clssq")
            act.activation(clssq[:].rearrange("p (k c) -> p k c", c=20), clsg,
                           ACT.Square, accum_out=acc[:, 2:3])

            # ---- reduce & out ----
            vec.tensor_reduce(out=acc[:, 0:1], in_=L, axis=X, op=AL.add)
            nc.sync.dma_start(out=out[:], in_=acc[:])
    nc.compile()
    return nc


def _get_nc():
    if "nc" not in _cache:
        _cache["nc"] = _build()
    return _cache["nc"]


def _host_prep(output, target):
    f32 = np.float32
    out_flat = output.reshape(CELLS, D)

    pt = np.zeros((CELLS, 32), dtype=BF16)
    pt[:, :D] = out_flat.astype(BF16)
    conf_all = np.ascontiguousarray(out_flat[:, 4:10:5]).astype(BF16)

    bid = target[:, 7].astype(np.int64)
    gx = target[:, 4].astype(np.int64)
    gy = target[:, 5].astype(np.int64)
    cell = bid * (GRID * GRID) + gx * GRID + gy

    order = np.argsort(cell, kind="stable")
    ts = target[order]
    cs = cell[order]
    core = cs // CELLS_CORE
    mod = cs % 4
    lp = ((cs % CELLS_CORE) // 4).astype(np.int16)

    x = ts[:, 0].astype(f32)
    y = ts[:, 1].astype(f32)
    w_ = ts[:, 2].astype(f32)
    h_ = ts[:, 3].astype(f32)
    c35 = f32(3.5)
    fields = np.empty((NTGT, 9), dtype=f32)
    fields[:, 0] = x
    fields[:, 1] = y
    fields[:, 2] = x - c35 * w_      # lef
    fields[:, 3] = y - c35 * h_      # top
    fields[:, 4] = x + c35 * w_      # rig
    fields[:, 5] = y + c35 * h_      # bot
    fields[:, 6] = np.sign(w_) * np.sqrt(np.abs(w_) + f32(1e-6))
    fields[:, 7] = np.sign(h_) * np.sqrt(np.abs(h_) + f32(1e-6))
    fields[:, 8] = (w_ * h_) * f32(49.0)
    clsid = ts[:, 6].astype(np.int64)
    hoh_all = np.zeros((NTGT, 20), dtype=f32)
    hoh_all[np.arange(NTGT), clsid] = f32(-2.0)

    in_maps = []
    for c in range(8):
        sel_c = core == c
        idxs = np.full(NS, ZROW, dtype=np.int16)
        fld = np.zeros((NS, 9), dtype=f32)
        hoh = np.zeros((NS, 20), dtype=f32)
        for m in range(NG):
            selm = sel_c & (mod == m)
            n = int(selm.sum())
            assert n <= CAP, f"group overflow: core {c} mod {m} n={n}"
            s0 = m * CAP
            idxs[s0:s0 + n] = lp[selm]
            fld[s0:s0 + n] = fields[selm]
            hoh[s0:s0 + n] = hoh_all[selm]

        idx16 = np.tile(idxs.reshape(IDXW, 16).T, (8, 1))          # [128, 608]
        pr = fld[:, :8].reshape(NCHUNK, P, 4, 2).transpose(1, 2, 0, 3)
        fldf = np.empty((P, 9 * NCHUNK), dtype=f32)
        fldf[:, :8 * NCHUNK] = pr.reshape(P, 8 * NCHUNK)
        fldf[:, 8 * NCHUNK:] = fld[:, 8].reshape(NCHUNK, P).T
        hcls = np.ascontiguousarray(
            hoh.reshape(NCHUNK, P, 20).transpose(1, 0, 2).reshape(P, 20 * NCHUNK)
        ).astype(BF16)
        win = np.concatenate(
            [pt[c * CELLS_CORE:(c + 1) * CELLS_CORE].reshape(PR_CORE, P),
             np.zeros((1, P), dtype=BF16)], axis=0)
        confc = np.ascontiguousarray(
            conf_all[c * CELLS_CORE:(c + 1) * CELLS_CORE]).reshape(P, CONF_W)
        in_maps.append({
            "win": np.ascontiguousarray(win),
            "idx": np.ascontiguousarray(idx16),
            "fldf": fldf,
            "hcls": hcls,
            "conf": confc,
        })
    return in_maps


def _reduce(results):
    tot = 0.0
    for res in results:
        p = res["partial"].astype(np.float64)
        tot += float(p[:, 0].sum()) + float(p[:, 2].sum()) + 0.5 * float(p[:, 1].sum())
    tot += NTGT - (8 * NS - NTGT)     # +1 per real target; -1 per padding slot
    return np.float32(tot)


def run(output, target, trace=False, trace_cores=None):
    from concourse.bass_utils import run_bass_kernel_spmd

    nc = _get_nc()
    in_maps = _host_prep(np.asarray(output), np.asarray(target))
    r = run_bass_kernel_spmd(nc, in_maps, core_ids=list(range(8)), trace=trace,
                             trace_cores=trace_cores)
    return _reduce(r.results), r


def kernel(output, target):
    return run(output, target)[0]


# revision 5
# speedup vs baseline: 1.1634x; 1.1634x over previous
"""YOLO-style loss (nn_Loss_90142773608781) on 8 Trainium2 NeuronCores.

Strategy (data-parallel by cell range, per sharding hint):
- Cells (16384*7*7 = 802816 rows of 30 floats) are sharded by batch range:
  core c owns cells [c*100352, (c+1)*100352).
- Dense conf term: host extracts cols {4,9} to a [CELLS,2] bf16 array; each
  core squares+accumulates its slice on ScalarE (one op).
- Targeted terms: targets are sharded by their cell's core. Grid rows are
  fetched with ONE dma_gather per core (fast CounterMachine SWDGE path,
  ~0.34ns/desc) from a bf16 table padded to 64B rows: each 256B gathered
  element covers 4 cells, and targets are grouped by cell%4 into 4
  fixed-capacity groups so the sub-offset is compile-time. Padding slots
  gather a dedicated all-zero row; their only residue, (0-1)^2 = 1 per slot,
  is corrected on the host.
- Class terms: host bakes -2*onehot(cls_t) so cls_r becomes a dot product;
  sum(cls^2) runs on ScalarE with accum. The per-target "+1" constant is
  added on the host.
- Each core writes [128,3] partials (target-loss, conf-sq, cls-sq); host
  reduces and applies the constant corrections.
"""

import sys

if "/opt/trn_rl_repo" not in sys.path:
    sys.path.append("/opt/trn_rl_repo")

import numpy as np
import ml_dtypes

BF16 = ml_dtypes.bfloat16

P = 128
D = 30
GRID = 7
BATCH = 16384
NTGT = 65536
CELLS = BATCH * GRID * GRID          # 802816
CELLS_CORE = CELLS // 8              # 100352
PR_CORE = CELLS_CORE // 4            # 25088 pair rows (4 cells / 256B elem)
ZROW = PR_CORE                       # dedicated zero row for padding slots
WINROWS = PR_CORE + 1
CAP = 2432                           # slots per (core, cell%4) group
NG = 4
NS = CAP * NG                        # 9728 slots per core
NCHUNK = NS // P                     # 76
GC = CAP // P                        # 19 chunks per group
IDXW = NS // 16                      # 608
CONF_W = CELLS_CORE * 2 // P         # 1568

_cache = {}


def _build():
    import concourse.bacc as bacc
    import concourse.tile as tile
    import concourse.mybir as mybir

    F32 = mybir.dt.float32
    BF = mybir.dt.bfloat16
    I16 = mybir.dt.int16
    AL = mybir.AluOpType
    ACT = mybir.ActivationFunctionType
    X = mybir.AxisListType.X

    nc = bacc.Bacc("TRN2", target_bir_lowering=False, debug=False,
                   enable_asserts=False, num_devices=8)
    win = nc.dram_tensor("win", [WINROWS, P], BF, kind="ExternalInput").ap()
    idx = nc.dram_tensor("idx", [P, IDXW], I16, kind="ExternalInput").ap()
    fldf = nc.dram_tensor("fldf", [P, 9 * NCHUNK], mybir.dt.float32,
                          kind="ExternalInput").ap()
    hcls = nc.dram_tensor("hcls", [P, 20 * NCHUNK], BF, kind="ExternalInput").ap()
    conf = nc.dram_tensor("conf", [P, CONF_W], BF, kind="ExternalInput").ap()
    out = nc.dram_tensor("partial", [P, 3], mybir.dt.float32,
                         kind="ExternalOutput").ap()

    vec, act = nc.vector, nc.scalar

    with tile.TileContext(nc) as tc:
        with (
            tc.tile_pool(name="io", bufs=1) as io,
            tc.tile_pool(name="scr", bufs=2) as scr,
        ):
            # ---- setup loads (idx first: gather depends on it) ----
            idx_t = io.tile([P, IDXW], I16)
            nc.sync.dma_start(out=idx_t[:], in_=idx[:])

            g = io.tile([P, NS], BF)
            g3v = g[:].rearrange("p (k e) -> p k e", e=P)
            n0 = 0
            while n0 < NS:
                n = min(1024, NS - n0)       # 64 descs/engine packet ceiling
                nc.gpsimd.dma_gather(
                    g3v[:, n0 // P:(n0 + n) // P, :],
                    win[:], idx_t[:, n0 // 16:(n0 + n) // 16], n, n, P,
                )
                n0 += n

            fld_t = io.tile([P, 9 * NCHUNK], F32)
            nc.sync.dma_start(out=fld_t[:], in_=fldf[:])
            h_t = io.tile([P, 20 * NCHUNK], BF)
            nc.sync.dma_start(out=h_t[:], in_=hcls[:])
            conf_t = io.tile([P, CONF_W], BF)
            nc.sync.dma_start(out=conf_t[:], in_=conf[:])

            eps_t = io.tile([P, 1], F32)
            vec.memset(eps_t[:], 1e-6)
            neg1_t = io.tile([P, 1], F32)
            vec.memset(neg1_t[:], -1.0)
            acc = io.tile([P, 3], F32)

            # ---- dense conf term on ScalarE (overlaps the gather) ----
            confsq = scr.tile([P, CONF_W], BF, tag="confsq")
            act.activation(confsq[:], conf_t[:], ACT.Square,
                           accum_out=acc[:, 1:2])

            # ---- consolidate gathered cells into packed f32 [P,76,30] ----
            G = io.tile([P, NCHUNK * D], F32)
            g3 = g[:].rearrange("p (k e) -> p k e", e=P)
            G3 = G[:].rearrange("p (k c) -> p k c", c=D)
            for m in range(NG):
                act.mul(G3[:, m * GC:(m + 1) * GC, :],
                        g3[:, m * GC:(m + 1) * GC, 32 * m:32 * m + D], 1.0)

            # ---- views ----
            W = NCHUNK
            g5 = G[:].rearrange("p (k b r) -> p k b r", b=6, r=5)
            xy = g5[:, :, 0:2, 0:2]          # [P,W,2box,2xy]
            wh = g5[:, :, 0:2, 2:4]
            cb = g5[:, :, 0:2, 4]            # [P,W,2]
            clsg = G3[:, :, 10:30]           # [P,W,20]

            pairs = fld_t[:, :8 * NCHUNK].rearrange("p (f k c) -> p f k c",
                                                    f=4, c=2)
            XYt = pairs[:, 0]                # [P,W,2]
            LTt = pairs[:, 1].unsqueeze(2).to_broadcast([P, W, 2, 2])
            RBt = pairs[:, 2].unsqueeze(2).to_broadcast([P, W, 2, 2])
            SSQt = pairs[:, 3]
            areab = fld_t[:, 8 * NCHUNK:9 * NCHUNK].rearrange(
                "p k -> p k").unsqueeze(2).to_broadcast([P, W, 2])

            def t4(tag):
                t = scr.tile([P, W * 4], F32, tag=tag, name=tag)
                return t[:].rearrange("p (k b r) -> p k b r", b=2, r=2)

            def t2(tag):
                t = scr.tile([P, W * 2], F32, tag=tag, name=tag)
                return t[:].rearrange("p (k c) -> p k c", c=2)

            def t1(tag):
                return scr.tile([P, W], F32, tag=tag, name=tag)[:]

            # ---- IoU / responsible-box selection ----
            hwh = t4("hwh")
            act.mul(hwh, wh, 3.5)
            lt = t4("lt")
            vec.tensor_tensor(out=lt, in0=xy, in1=hwh, op=AL.subtract)
            rb = t4("rb")
            vec.tensor_tensor(out=rb, in0=xy, in1=hwh, op=AL.add)

            wih = t4("wih")
            vec.tensor_tensor(out=wih, in0=rb, in1=RBt, op=AL.min)
            mx = t4("mx")
            vec.tensor_tensor(out=mx, in0=lt, in1=LTt, op=AL.max)
            vec.tensor_tensor(out=wih, in0=wih, in1=mx, op=AL.subtract)
            vec.tensor_scalar_max(out=wih, in0=wih, scalar1=0.0)

            ain = t2("ain")
            vec.tensor_tensor(out=ain, in0=wih[:, :, :, 0], in1=wih[:, :, :, 1],
                              op=AL.mult)
            atot = t2("atot")
            vec.tensor_tensor(out=atot, in0=wh[:, :, :, 0], in1=wh[:, :, :, 1],
                              op=AL.mult)
            act.mul(atot, atot, 49.0)
            vec.tensor_tensor(out=atot, in0=atot, in1=areab, op=AL.add)
            vec.tensor_tensor(out=atot, in0=atot, in1=ain, op=AL.subtract)

            pred = t2("pred")
            vec.tensor_scalar(out=pred, in0=atot, scalar1=1e-6, scalar2=None,
                              op0=AL.is_gt)
            vec.tensor_scalar_max(out=atot, in0=atot, scalar1=1e-6)
            vec.reciprocal(out=atot, in_=atot)
            iou = t2("iou")
            vec.tensor_tensor(out=iou, in0=ain, in1=atot, op=AL.mult)
            vec.tensor_tensor(out=iou, in0=iou, in1=pred, op=AL.mult)

            sel2 = t2("sel2")
            i1 = iou[:, :, 1:2].to_broadcast([P, W, 2])
            i0 = iou[:, :, 0:1].to_broadcast([P, W, 2])
            vec.tensor_tensor(out=sel2, in0=i1, in1=i0, op=AL.is_gt)

            def pick2(v3, tag):
                t = t2(tag)
                vec.tensor_tensor(out=t, in0=v3[:, :, 1, :], in1=v3[:, :, 0, :],
                                  op=AL.subtract)
                vec.tensor_tensor(out=t, in0=t, in1=sel2, op=AL.mult)
                vec.tensor_tensor(out=t, in0=t, in1=v3[:, :, 0, :], op=AL.add)
                return t

            xyr = pick2(xy, "xyr")
            whr = pick2(wh, "whr")
            cr = t1("cr")
            vec.tensor_tensor(out=cr, in0=cb[:, :, 1], in1=cb[:, :, 0],
                              op=AL.subtract)
            vec.tensor_tensor(out=cr, in0=cr, in1=sel2[:, :, 0], op=AL.mult)
            vec.tensor_tensor(out=cr, in0=cr, in1=cb[:, :, 0], op=AL.add)

            # ---- coord + size terms ----
            dxy = t2("dxy")
            vec.tensor_tensor(out=dxy, in0=XYt, in1=xyr, op=AL.subtract)
            vec.tensor_tensor(out=dxy, in0=dxy, in1=dxy, op=AL.mult)

            sq_ = t2("sq_")
            sg_ = t2("sg_")
            act.activation(sq_, whr, ACT.Abs)
            act.activation(sq_, sq_, ACT.Sqrt, bias=eps_t[:])
            act.activation(sg_, whr, ACT.Sign)
            vec.tensor_tensor(out=sq_, in0=sq_, in1=sg_, op=AL.mult)
            vec.tensor_tensor(out=sq_, in0=SSQt, in1=sq_, op=AL.subtract)
            vec.tensor_tensor(out=sq_, in0=sq_, in1=sq_, op=AL.mult)

            vec.tensor_tensor(out=dxy, in0=dxy, in1=sq_, op=AL.add)
            L = t1("L")
            vec.tensor_reduce(out=L, in_=dxy, axis=X, op=AL.add)
            act.mul(L, L, 5.0)

            # ---- obj terms: + (cr-1)^2 - 0.5*cr^2 ----
            o1 = t1("o1")
            act.activation(o1, cr, ACT.Square, bias=neg1_t[:])
            vec.tensor_tensor(out=L, in0=L, in1=o1, op=AL.add)
            act.activation(o1, cr, ACT.Square)
            vec.tensor_scalar_mul(out=o1, in0=o1, scalar1=0.5)
            vec.tensor_tensor(out=L, in0=L, in1=o1, op=AL.subtract)

            # ---- class terms: -2*cls_sel (host-baked one-hot) + cls^2 ----
            big = scr.tile([P, W * 20], F32, tag="big")
            big3 = big[:].rearrange("p (k c) -> p k c", c=20)
            h3 = h_t[:].rearrange("p (k c) -> p k c", c=20)
            vec.tensor_tensor(out=big3, in0=h3, in1=clsg, op=AL.mult)
            vec.tensor_reduce(out=o1, in_=big3, axis=X, op=AL.add)
            vec.tensor_tensor(out=L, in0=L, in1=o1, op=AL.add)

            clssq = scr.tile([P, W * 20], BF, tag="clssq")
            act.activation(clssq[:].rearrange("p (k c) -> p k c", c=20), clsg,
                           ACT.Square, accum_out=acc[:, 2:3])

            # ---- reduce & out ----
            vec.tensor_reduce(out=acc[:, 0:1], in_=L, axis=X, op=AL.add)
            nc.sync.dma_start(out=out[:], in_=acc[:])
    nc.compile()
    return nc


def _get_nc():
    if "nc" not in _cache:
        _cache["nc"] = _build()
    return _cache["nc"]


def _host_prep(output, target):
    f32 = np.float32
    out_flat = output.reshape(CELLS, D)

    pt = np.zeros((CELLS, 32), dtype=BF16)
    pt[:, :D] = out_flat.astype(BF16)
    conf_all = np.ascontiguousarray(out_flat[:, 4:10:5]).astype(BF16)

    bid = target[:, 7].astype(np.int64)
    gx = target[:, 4].astype(np.int64)
    gy = target[:, 5].astype(np.int64)
    cell = bid * (GRID * GRID) + gx * GRID + gy

    order = np.argsort(cell, kind="stable")
    ts = target[order]
    cs = cell[order]
    core = cs // CELLS_CORE
    mod = cs % 4
    lp = ((cs % CELLS_CORE) // 4).astype(np.int16)

    x = ts[:, 0].astype(f32)
    y = ts[:, 1].astype(f32)
    w_ = ts[:, 2].astype(f32)
    h_ = ts[:, 3].astype(f32)
    c35 = f32(3.5)
    fields = np.empty((NTGT, 9), dtype=f32)
    fields[:, 0] = x
    fields[:, 1] = y
    fields[:, 2] = x - c35 * w_      # lef
    fields[:, 3] = y - c35 * h_      # top
    fields[:, 4] = x + c35 * w_      # rig
    fields[:, 5] = y + c35 * h_      # bot
    fields[:, 6] = np.sign(w_) * np.sqrt(np.abs(w_) + f32(1e-6))
    fields[:, 7] = np.sign(h_) * np.sqrt(np.abs(h_) + f32(1e-6))
    fields[:, 8] = (w_ * h_) * f32(49.0)
    clsid = ts[:, 6].astype(np.int64)
    hoh_all = np.zeros((NTGT, 20), dtype=f32)
    hoh_all[np.arange(NTGT), clsid] = f32(-2.0)

    in_maps = []
    for c in range(8):
        sel_c = core == c
        idxs = np.full(NS, ZROW, dtype=np.int16)
        fld = np.zeros((NS, 9), dtype=f32)
        hoh = np.zeros((NS, 20), dtype=f32)
        for m in range(NG):
            selm = sel_c & (mod == m)
            n = int(selm.sum())
            assert n <= CAP, f"group overflow: core {c} mod {m} n={n}"
            s0 = m * CAP
            idxs[s0:s0 + n] = lp[selm]
            fld[s0:s0 + n] = fields[selm]
            hoh[s0:s0 + n] = hoh_all[selm]

        idx16 = np.tile(idxs.reshape(IDXW, 16).T, (8, 1))          # [128, 608]
        pr = fld[:, :8].reshape(NCHUNK, P, 4, 2).transpose(1, 2, 0, 3)
        fldf = np.empty((P, 9 * NCHUNK), dtype=f32)
        fldf[:, :8 * NCHUNK] = pr.reshape(P, 8 * NCHUNK)
        fldf[:, 8 * NCHUNK:] = fld[:, 8].reshape(NCHUNK, P).T
        hcls = np.ascontiguousarray(
            hoh.reshape(NCHUNK, P, 20).transpose(1, 0, 2).reshape(P, 20 * NCHUNK)
        ).astype(BF16)
        win = np.concatenate(
            [pt[c * CELLS_CORE:(c + 1) * CELLS_CORE].reshape(PR_CORE, P),
             np.zeros((1, P), dtype=BF16)], axis=0)
        confc = np.ascontiguousarray(
            conf_all[c * CELLS_CORE:(c + 1) * CELLS_CORE]).reshape(P, CONF_W)
        in_maps.append({
            "win": np.ascontiguousarray(win),
            "idx": np.ascontiguousarray(idx16),
            "fldf": fldf,
            "hcls": hcls,
            "conf": confc,
        })
    return in_maps


def _reduce(results):
    tot = 0.0
    for res in results:
        p = res["partial"].astype(np.float64)
        tot += float(p[:, 0].sum()) + float(p[:, 2].sum()) + 0.5 * float(p[:, 1].sum())
    tot += NTGT - (8 * NS - NTGT)     # +1 per real target; -1 per padding slot
    return np.float32(tot)


def run(output, target, trace=False, trace_cores=None):
    from concourse.bass_utils import run_bass_kernel_spmd

    nc = _get_nc()
    in_maps = _host_prep(np.asarray(output), np.asarray(target))
    r = run_bass_kernel_spmd(nc, in_maps, core_ids=list(range(8)), trace=trace,
                             trace_cores=trace_cores)
    return _reduce(r.results), r


def kernel(output, target):
    return run(output, target)[0]


# revision 7
# speedup vs baseline: 1.9462x; 1.6729x over previous
"""YOLO-style loss (nn_Loss_90142773608781) on 8 Trainium2 NeuronCores.

Strategy (data-parallel by cell range, per sharding hint):
- Cells (16384*7*7 = 802816 rows of 30 floats) are sharded by batch range:
  core c owns cells [c*100352, (c+1)*100352).
- Dense conf term: host extracts cols {4,9} to a [CELLS,2] bf16 array; each
  core squares+accumulates its slice on ScalarE (one op).
- Targeted terms: targets are sharded by their cell's core. Grid rows are
  fetched with ONE dma_gather per core (fast CounterMachine SWDGE path,
  ~0.34ns/desc) from a bf16 table padded to 64B rows: each 256B gathered
  element covers 4 cells, and targets are grouped by cell%4 into 4
  fixed-capacity groups so the sub-offset is compile-time. Padding slots
  gather a dedicated all-zero row; their only residue, (0-1)^2 = 1 per slot,
  is corrected on the host.
- Class terms: host bakes -2*onehot(cls_t) so cls_r becomes a dot product;
  sum(cls^2) runs on ScalarE with accum. The per-target "+1" constant is
  added on the host.
- Each core writes [128,3] partials (target-loss, conf-sq, cls-sq); host
  reduces and applies the constant corrections.
"""

import sys

if "/opt/trn_rl_repo" not in sys.path:
    sys.path.append("/opt/trn_rl_repo")

import numpy as np
import ml_dtypes

BF16 = ml_dtypes.bfloat16

P = 128
D = 30
GRID = 7
BATCH = 16384
NTGT = 65536
CELLS = BATCH * GRID * GRID          # 802816
CELLS_CORE = CELLS // 8              # 100352
PR_CORE = CELLS_CORE // 4            # 25088 pair rows (4 cells / 256B elem)
ZROW = PR_CORE                       # dedicated zero row for padding slots
WINROWS = PR_CORE + 1
CAP = 2432                           # slots per (core, cell%4) group
NG = 4
NS = CAP * NG                        # 9728 slots per core
NCHUNK = NS // P                     # 76
GC = CAP // P                        # 19 chunks per group
IDXW = NS // 16                      # 608
CONF_W = CELLS_CORE * 2 // P         # 1568

_cache = {}


def _build():
    import concourse.bacc as bacc
    import concourse.tile as tile
    import concourse.mybir as mybir

    F32 = mybir.dt.float32
    BF = mybir.dt.bfloat16
    I16 = mybir.dt.int16
    AL = mybir.AluOpType
    ACT = mybir.ActivationFunctionType
    X = mybir.AxisListType.X

    nc = bacc.Bacc("TRN2", target_bir_lowering=False, debug=False,
                   enable_asserts=False, num_devices=8, num_swdge_queues=4)
    win = nc.dram_tensor("win", [WINROWS, P], BF, kind="ExternalInput").ap()
    idx = nc.dram_tensor("idx", [P, IDXW], I16, kind="ExternalInput").ap()
    fldf = nc.dram_tensor("fldf", [P, 9 * NCHUNK], mybir.dt.float32,
                          kind="ExternalInput").ap()
    hcls = nc.dram_tensor("hcls", [P, 20 * NCHUNK], BF, kind="ExternalInput").ap()
    conf = nc.dram_tensor("conf", [P, CONF_W], BF, kind="ExternalInput").ap()
    out = nc.dram_tensor("partial", [P, 3], mybir.dt.float32,
                         kind="ExternalOutput").ap()

    vec, act = nc.vector, nc.scalar

    with tile.TileContext(nc) as tc:
        with (
            tc.tile_pool(name="io", bufs=1) as io,
            tc.tile_pool(name="scr", bufs=2) as scr,
        ):
            # ---- setup loads (idx first: gather depends on it) ----
            idx_t = io.tile([P, IDXW], I16)
            nc.sync.dma_start(out=idx_t[:], in_=idx[:])

            g = io.tile([P, NS], BF)
            g3v = g[:].rearrange("p (k e) -> p k e", e=P)
            n0 = 0
            qi = 0
            while n0 < NS:
                n = min(1024, NS - n0)       # 64 descs/engine packet ceiling
                nc.gpsimd.dma_gather(
                    g3v[:, n0 // P:(n0 + n) // P, :],
                    win[:], idx_t[:, n0 // 16:(n0 + n) // 16], n, n, P,
                    queue_num=qi % 4,
                )
                qi += 1
                n0 += n

            fld_t = io.tile([P, 9 * NCHUNK], F32)
            nc.sync.dma_start(out=fld_t[:], in_=fldf[:])
            h_t = io.tile([P, 20 * NCHUNK], BF)
            nc.sync.dma_start(out=h_t[:], in_=hcls[:])
            conf_t = io.tile([P, CONF_W], BF)
            nc.sync.dma_start(out=conf_t[:], in_=conf[:])

            eps_t = io.tile([P, 1], F32)
            vec.memset(eps_t[:], 1e-6)
            neg1_t = io.tile([P, 1], F32)
            vec.memset(neg1_t[:], -1.0)
            acc = io.tile([P, 3], F32)

            # ---- dense conf term on ScalarE (overlaps the gather) ----
            confsq = scr.tile([P, CONF_W], BF, tag="confsq")
            act.activation(confsq[:], conf_t[:], ACT.Square,
                           accum_out=acc[:, 1:2])

            # ---- consolidate gathered cells into packed f32 [P,76,30] ----
            G = io.tile([P, NCHUNK * D], F32)
            g3 = g[:].rearrange("p (k e) -> p k e", e=P)
            G3 = G[:].rearrange("p (k c) -> p k c", c=D)
            for m in range(NG):
                act.mul(G3[:, m * GC:(m + 1) * GC, :],
                        g3[:, m * GC:(m + 1) * GC, 32 * m:32 * m + D], 1.0)

            # ---- views ----
            W = NCHUNK
            g5 = G[:].rearrange("p (k b r) -> p k b r", b=6, r=5)
            xy = g5[:, :, 0:2, 0:2]          # [P,W,2box,2xy]
            wh = g5[:, :, 0:2, 2:4]
            cb = g5[:, :, 0:2, 4]            # [P,W,2]
            clsg = G3[:, :, 10:30]           # [P,W,20]

            pairs = fld_t[:, :8 * NCHUNK].rearrange("p (f k c) -> p f k c",
                                                    f=4, c=2)
            XYt = pairs[:, 0]                # [P,W,2]
            LTt = pairs[:, 1].unsqueeze(2).to_broadcast([P, W, 2, 2])
            RBt = pairs[:, 2].unsqueeze(2).to_broadcast([P, W, 2, 2])
            SSQt = pairs[:, 3]
            areab = fld_t[:, 8 * NCHUNK:9 * NCHUNK].rearrange(
                "p k -> p k").unsqueeze(2).to_broadcast([P, W, 2])

            def t4(tag):
                t = scr.tile([P, W * 4], F32, tag=tag, name=tag)
                return t[:].rearrange("p (k b r) -> p k b r", b=2, r=2)

            def t2(tag):
                t = scr.tile([P, W * 2], F32, tag=tag, name=tag)
                return t[:].rearrange("p (k c) -> p k c", c=2)

            def t1(tag):
                return scr.tile([P, W], F32, tag=tag, name=tag)[:]

            # ---- IoU / responsible-box selection ----
            hwh = t4("hwh")
            act.mul(hwh, wh, 3.5)
            lt = t4("lt")
            vec.tensor_tensor(out=lt, in0=xy, in1=hwh, op=AL.subtract)
            rb = t4("rb")
            vec.tensor_tensor(out=rb, in0=xy, in1=hwh, op=AL.add)

            wih = t4("wih")
            vec.tensor_tensor(out=wih, in0=rb, in1=RBt, op=AL.min)
            mx = t4("mx")
            vec.tensor_tensor(out=mx, in0=lt, in1=LTt, op=AL.max)
            vec.tensor_tensor(out=wih, in0=wih, in1=mx, op=AL.subtract)
            vec.tensor_scalar_max(out=wih, in0=wih, scalar1=0.0)

            ain = t2("ain")
            vec.tensor_tensor(out=ain, in0=wih[:, :, :, 0], in1=wih[:, :, :, 1],
                              op=AL.mult)
            atot = t2("atot")
            vec.tensor_tensor(out=atot, in0=wh[:, :, :, 0], in1=wh[:, :, :, 1],
                              op=AL.mult)
            act.mul(atot, atot, 49.0)
            vec.tensor_tensor(out=atot, in0=atot, in1=areab, op=AL.add)
            vec.tensor_tensor(out=atot, in0=atot, in1=ain, op=AL.subtract)

            pred = t2("pred")
            vec.tensor_scalar(out=pred, in0=atot, scalar1=1e-6, scalar2=None,
                              op0=AL.is_gt)
            vec.tensor_scalar_max(out=atot, in0=atot, scalar1=1e-6)
            vec.reciprocal(out=atot, in_=atot)
            iou = t2("iou")
            vec.tensor_tensor(out=iou, in0=ain, in1=atot, op=AL.mult)
            vec.tensor_tensor(out=iou, in0=iou, in1=pred, op=AL.mult)

            sel2 = t2("sel2")
            i1 = iou[:, :, 1:2].to_broadcast([P, W, 2])
            i0 = iou[:, :, 0:1].to_broadcast([P, W, 2])
            vec.tensor_tensor(out=sel2, in0=i1, in1=i0, op=AL.is_gt)

            def pick2(v3, tag):
                t = t2(tag)
                vec.tensor_tensor(out=t, in0=v3[:, :, 1, :], in1=v3[:, :, 0, :],
                                  op=AL.subtract)
                vec.tensor_tensor(out=t, in0=t, in1=sel2, op=AL.mult)
                vec.tensor_tensor(out=t, in0=t, in1=v3[:, :, 0, :], op=AL.add)
                return t

            xyr = pick2(xy, "xyr")
            whr = pick2(wh, "whr")
            cr = t1("cr")
            vec.tensor_tensor(out=cr, in0=cb[:, :, 1], in1=cb[:, :, 0],
                              op=AL.subtract)
            vec.tensor_tensor(out=cr, in0=cr, in1=sel2[:, :, 0], op=AL.mult)
            vec.tensor_tensor(out=cr, in0=cr, in1=cb[:, :, 0], op=AL.add)

            # ---- coord + size terms ----
            dxy = t2("dxy")
            vec.tensor_tensor(out=dxy, in0=XYt, in1=xyr, op=AL.subtract)
            vec.tensor_tensor(out=dxy, in0=dxy, in1=dxy, op=AL.mult)

            sq_ = t2("sq_")
            sg_ = t2("sg_")
            act.activation(sq_, whr, ACT.Abs)
            act.activation(sq_, sq_, ACT.Sqrt, bias=eps_t[:])
            act.activation(sg_, whr, ACT.Sign)
            vec.tensor_tensor(out=sq_, in0=sq_, in1=sg_, op=AL.mult)
            vec.tensor_tensor(out=sq_, in0=SSQt, in1=sq_, op=AL.subtract)
            vec.tensor_tensor(out=sq_, in0=sq_, in1=sq_, op=AL.mult)

            vec.tensor_tensor(out=dxy, in0=dxy, in1=sq_, op=AL.add)
            L = t1("L")
            vec.tensor_reduce(out=L, in_=dxy, axis=X, op=AL.add)
            act.mul(L, L, 5.0)

            # ---- obj terms: + (cr-1)^2 - 0.5*cr^2 ----
            o1 = t1("o1")
            act.activation(o1, cr, ACT.Square, bias=neg1_t[:])
            vec.tensor_tensor(out=L, in0=L, in1=o1, op=AL.add)
            act.activation(o1, cr, ACT.Square)
            vec.tensor_scalar_mul(out=o1, in0=o1, scalar1=0.5)
            vec.tensor_tensor(out=L, in0=L, in1=o1, op=AL.subtract)

            # ---- class terms: -2*cls_sel (host-baked one-hot) + cls^2 ----
            big = scr.tile([P, W * 20], F32, tag="big")
            big3 = big[:].rearrange("p (k c) -> p k c", c=20)
            h3 = h_t[:].rearrange("p (k c) -> p k c", c=20)
            vec.tensor_tensor(out=big3, in0=h3, in1=clsg, op=AL.mult)
            vec.tensor_reduce(out=o1, in_=big3, axis=X, op=AL.add)
            vec.tensor_tensor(out=L, in0=L, in1=o1, op=AL.add)

            clssq = scr.tile([P, W * 20], BF, tag="clssq")
            act.activation(clssq[:].rearrange("p (k c) -> p k c", c=20), clsg,
                           ACT.Square, accum_out=acc[:, 2:3])

            # ---- reduce & out ----
            vec.tensor_reduce(out=acc[:, 0:1], in_=L, axis=X, op=AL.add)
            nc.sync.dma_start(out=out[:], in_=acc[:])
    nc.compile()
    return nc


def _get_nc():
    if "nc" not in _cache:
        _cache["nc"] = _build()
    return _cache["nc"]


def _host_prep(output, target):
    f32 = np.float32
    out_flat = output.reshape(CELLS, D)

    pt = np.zeros((CELLS, 32), dtype=BF16)
    pt[:, :D] = out_flat.astype(BF16)
    conf_all = np.ascontiguousarray(out_flat[:, 4:10:5]).astype(BF16)

    bid = target[:, 7].astype(np.int64)
    gx = target[:, 4].astype(np.int64)
    gy = target[:, 5].astype(np.int64)
    cell = bid * (GRID * GRID) + gx * GRID + gy

    order = np.argsort(cell, kind="stable")
    ts = target[order]
    cs = cell[order]
    core = cs // CELLS_CORE
    mod = cs % 4
    lp = ((cs % CELLS_CORE) // 4).astype(np.int16)

    x = ts[:, 0].astype(f32)
    y = ts[:, 1].astype(f32)
    w_ = ts[:, 2].astype(f32)
    h_ = ts[:, 3].astype(f32)
    c35 = f32(3.5)
    fields = np.empty((NTGT, 9), dtype=f32)
    fields[:, 0] = x
    fields[:, 1] = y
    fields[:, 2] = x - c35 * w_      # lef
    fields[:, 3] = y - c35 * h_      # top
    fields[:, 4] = x + c35 * w_      # rig
    fields[:, 5] = y + c35 * h_      # bot
    fields[:, 6] = np.sign(w_) * np.sqrt(np.abs(w_) + f32(1e-6))
    fields[:, 7] = np.sign(h_) * np.sqrt(np.abs(h_) + f32(1e-6))
    fields[:, 8] = (w_ * h_) * f32(49.0)
    clsid = ts[:, 6].astype(np.int64)
    hoh_all = np.zeros((NTGT, 20), dtype=f32)
    hoh_all[np.arange(NTGT), clsid] = f32(-2.0)

    in_maps = []
    for c in range(8):
        sel_c = core == c
        idxs = np.full(NS, ZROW, dtype=np.int16)
        fld = np.zeros((NS, 9), dtype=f32)
        hoh = np.zeros((NS, 20), dtype=f32)
        for m in range(NG):
            selm = sel_c & (mod == m)
            n = int(selm.sum())
            assert n <= CAP, f"group overflow: core {c} mod {m} n={n}"
            s0 = m * CAP
            idxs[s0:s0 + n] = lp[selm]
            fld[s0:s0 + n] = fields[selm]
            hoh[s0:s0 + n] = hoh_all[selm]

        idx16 = np.tile(idxs.reshape(IDXW, 16).T, (8, 1))          # [128, 608]
        pr = fld[:, :8].reshape(NCHUNK, P, 4, 2).transpose(1, 2, 0, 3)
        fldf = np.empty((P, 9 * NCHUNK), dtype=f32)
        fldf[:, :8 * NCHUNK] = pr.reshape(P, 8 * NCHUNK)
        fldf[:, 8 * NCHUNK:] = fld[:, 8].reshape(NCHUNK, P).T
        hcls = np.ascontiguousarray(
            hoh.reshape(NCHUNK, P, 20).transpose(1, 0, 2).reshape(P, 20 * NCHUNK)
        ).astype(BF16)
        win = np.concatenate(
            [pt[c * CELLS_CORE:(c + 1) * CELLS_CORE].reshape(PR_CORE, P),
             np.zeros((1, P), dtype=BF16)], axis=0)
        confc = np.ascontiguousarray(
            conf_all[c * CELLS_CORE:(c + 1) * CELLS_CORE]).reshape(P, CONF_W)
        in_maps.append({
            "win": np.ascontiguousarray(win),
            "idx": np.ascontiguousarray(idx16),
            "fldf": fldf,
            "hcls": hcls,
            "conf": confc,
        })
    return in_maps


def _reduce(results):
    tot = 0.0
    for res in results:
        p = res["partial"].astype(np.float64)
        tot += float(p[:, 0].sum()) + float(p[:, 2].sum()) + 0.5 * float(p[:, 1].sum())
    tot += NTGT - (8 * NS - NTGT)     # +1 per real target; -1 per padding slot
    return np.float32(tot)


def run(output, target, trace=False, trace_cores=None):
    from concourse.bass_utils import run_bass_kernel_spmd

    nc = _get_nc()
    in_maps = _host_prep(np.asarray(output), np.asarray(target))
    r = run_bass_kernel_spmd(nc, in_maps, core_ids=list(range(8)), trace=trace,
                             trace_cores=trace_cores)
    return _reduce(r.results), r


def kernel(output, target):
    return run(output, target)[0]
